# revision 1
# baseline (speedup 1.0000x reference)
"""CGCNN (no BN) message-passing GNN on 8 Trainium2 NeuronCores.

Strategy (self-contained; shapes hardcoded from the problem spec):
 - Nodes are permuted on the host into 392 blocks of 128 slots, balancing
   per-block in-edge counts. Cores own 49 contiguous blocks (6272 slots).
 - Edges are owned by the core that owns their destination block; within a
   block, edges are split by source-slot half (<32768 vs >=32768) so each
   128-edge tile gathers from a single int16-indexable table view, then
   padded to a uniform (TL, TH) tile count per block so all 8 cores run one
   SPMD program.
 - Host->device traffic is minimized (the axon relay is ~30 MB/s):
     * edge features (gaussian smearing) are computed ON DEVICE from a
       [2, S] f32 (d^2, d) array via a K=2 matmul + Exp, cached in DRAM;
     * the full bf16 gather table AND the f32 x0 residual table are built
       ON DEVICE by dma_gather of embedding rows indexed by z;
     * gather index arrays are shipped once, un-replicated ([16, n/16]) and
       expanded to the 128-partition layout on device;
     * LN params / iota matrices are built on device from row vectors;
     * per-graph mean-pooling runs on device (one-hot matmuls accumulated
       in PSUM over the last conv layer), so each core returns only a
       [256, 128] f32 partial sum.
 - Per 128-edge tile on device: dma_gather (SBUF source, transposed) pulls
   x[src] / x[dst] columns in channel-major bf16; three PE matmuls
   (xi@W1 + xj@W2 + eaT@W3b) accumulate the conv pre-activation in PSUM;
   ACT computes sigmoid/softplus; DVE builds a one-hot dst matrix which PE
   uses to segment-sum messages into the block accumulator.
 - LayerNorm + residual + softplus per block in f32; updated x is written to
   a bf16 local table (for x[dst] gathers) and, between conv layers, an
   in-kernel AllGather replicates every core's slice into the full bf16
   gather table.
 - The tiny pooled-MLP head runs on the host in f32.
"""

import os as _os
import numpy as np
import ml_dtypes

import concourse.bass as bass
import concourse.tile as tile
from concourse import bacc, mybir

BF16 = ml_dtypes.bfloat16

# Problem constants
N_NODES, N_EDGES, NODE_D, EDGE_D, EMB_D, N_GRAPHS = 50000, 800000, 128, 100, 92, 256
N_CONV, FC_D, N_FC, CUTOFF = 3, 128, 2, 6.0

LAST_RESULTS = None        # BassKernelResults of the most recent run (for tests)
LAST_RERUN_S = None        # wall seconds of a warm re-execution

N_CORES = 8
UQMAX = 6.625              # d quantization range (beyond it all gaussians ~ 0)
SLOTS = 50176              # 392 blocks * 128
BLOCKS = SLOTS // 128      # 392
NBLK = BLOCKS // N_CORES   # 49 blocks per core
CORE_SLOTS = NBLK * 128    # 6272
LO_SLOTS = 32768           # slots gatherable from the low table view
CHUNK = 2                  # blocks processed per gather chunk

_NC_CACHE = {}


# --------------------------------------------------------------------------
# Input blob layout (shared between host packing and device program)
# --------------------------------------------------------------------------

def _layout(TL, TH, nblk=NBLK, ranks=BLOCKS, n_cores=N_CORES):
    """Byte layout of the per-core-unique and shared input blobs.

    Returns (uents, sents, BU, BS, BS8): entry dicts name -> (off, shape,
    np_dtype), blob sizes in bytes. All offsets 512-aligned; BS is a
    multiple of n_cores*512 so the shared blob splits evenly.
    """
    TPB = TL + TH
    NT = nblk * TPB
    S = NT * 128
    SLO = nblk * TL * 128
    SHI = nblk * TH * 128
    slots = ranks * 128
    core_slots = nblk * 128

    uspec = [
        ("u", (1, S), np.int16),        # d quantized to [0, UQMAX]
        ("ixi", (16, S // 16), np.uint8),
        ("ixlo", (16, SLO // 16), np.int16),
        ("ixhi", (16, SHI // 16), np.int16),
        ("zown", (16, core_slots // 16), np.int16),
        ("dstv", (128, NT), np.int8),
        ("gid", (128, nblk), np.float32),
    ]
    sspec = [
        ("zall", (16, slots // 16), np.int16),
        ("ewb", (128, 128), BF16),
        ("ewf", (128, 128), np.float32),
        ("wxi", (128, N_CONV, 256), BF16),
        ("wxj", (128, N_CONV, 256), BF16),
        ("wea", (101, N_CONV, 256), BF16),
        ("lnr", (1, N_CONV * 256), np.float32),
        ("ior", (1, 256), np.float32),
        ("noffs", (101, 1), np.float32),
        ("cfs", (101, 1), np.float32),
    ]

    def place(spec):
        ents, off = {}, 0
        for name, shape, dt_ in spec:
            nbytes = int(np.prod(shape)) * np.dtype(dt_).itemsize
            ents[name] = (off, shape, dt_)
            off += -(-nbytes // 512) * 512
        return ents, off

    uents, BU = place(uspec)
    sents, BS = place(sspec)
    BU = -(-BU // 512) * 512
    align = n_cores * 512
    BS = -(-BS // align) * align
    return uents, sents, BU, BS, BS // n_cores


_NP2MY = None


def _my_dt(np_dt):
    global _NP2MY
    if _NP2MY is None:
        _NP2MY = {np.dtype(np.float32): mybir.dt.float32,
                  np.dtype(np.int16): mybir.dt.int16,
                  np.dtype(np.int8): mybir.dt.int8,
                  np.dtype(np.uint8): mybir.dt.uint8,
                  np.dtype(BF16): mybir.dt.bfloat16}
    return _NP2MY[np.dtype(np_dt)]


# --------------------------------------------------------------------------
# Device program
# --------------------------------------------------------------------------

def build_nc(TL, TH, nblk=NBLK, ranks=BLOCKS, n_cores=N_CORES,
             lo_ranks=LO_SLOTS // 128, debug_tabs=False):
    """Build the SPMD Bass program. TL/TH = low/high tiles per block."""
    TPB = TL + TH                 # tiles per block
    NT = nblk * TPB               # tiles per core
    S = NT * 128                  # edge slots per core
    SLO = nblk * TL * 128
    SHI = nblk * TH * 128
    slots = ranks * 128
    core_slots = nblk * 128
    f32, bf, i16 = mybir.dt.float32, mybir.dt.bfloat16, mybir.dt.int16
    AF = mybir.ActivationFunctionType

    nc = bacc.Bacc("TRN2", target_bir_lowering=False, debug=False,
                   num_devices=n_cores)

    # ---- external inputs: two flat blobs ---------------------------------
    uents, sents, BU, BS, BS8 = _layout(TL, TH, nblk, ranks, n_cores)
    ublob_d = nc.dram_tensor("ublob", [BU // 2], i16, kind="ExternalInput").ap()
    sblob_d = nc.dram_tensor("sblob", [BS8 // 2], i16, kind="ExternalInput").ap()
    sstage_d = nc.dram_tensor("sstage", [BS8 // 2], i16, kind="Internal").ap()
    sall_d = nc.dram_tensor("sall", [BS // 2], i16, kind="Internal",
                            addr_space="Shared").ap()

    def V(ents, base):
        def view(name):
            off, shape, dt_ = ents[name]
            nbytes = int(np.prod(shape)) * np.dtype(dt_).itemsize
            v = base[off // 2: (off + nbytes) // 2].bitcast(_my_dt(dt_))
            if len(shape) == 2:
                v = v.rearrange("(a b) -> a b", b=shape[1])
            elif len(shape) == 3:
                v = v.rearrange("(a b c) -> a b c", b=shape[1], c=shape[2])
            return v
        return view

    UV = V(uents, ublob_d)
    SV = V(sents, sall_d)
    u_d = UV("u")
    ixi_d, ixlo_d, ixhi_d = UV("ixi"), UV("ixlo"), UV("ixhi")
    zown_d, dst_d, gid_d = UV("zown"), UV("dstv"), UV("gid")
    zall_d, ewb_d, ewf_d = SV("zall"), SV("ewb"), SV("ewf")
    wxi_d, wxj_d, wea_d = SV("wxi"), SV("wxj"), SV("wea")
    lnr_d, ior_d = SV("lnr"), SV("ior")
    noffs_d, cfs_d = SV("noffs"), SV("cfs")

    # ---- internal DRAM ---------------------------------------------------
    ea_h = nc.dram_tensor("ea_h", [101, S], bf, kind="Internal").ap()
    ixi_x = nc.dram_tensor("ixi_x", [128, S // 16], i16, kind="Internal").ap()
    ixlo_x = nc.dram_tensor("ixlo_x", [128, SLO // 16], i16, kind="Internal").ap()
    ixhi_x = nc.dram_tensor("ixhi_x", [128, SHI // 16], i16, kind="Internal").ap()
    zall_x = nc.dram_tensor("zall_x", [128, slots // 16], i16, kind="Internal").ap()
    zown_x = nc.dram_tensor("zown_x", [128, core_slots // 16], i16,
                            kind="Internal").ap()
    xinit = nc.dram_tensor("xinit", [core_slots, 128], f32, kind="Internal").ap()
    xmast = [
        nc.dram_tensor(f"xmast{i}", [core_slots, 128], f32, kind="Internal").ap()
        for i in range(2)
    ]
    xout = [
        nc.dram_tensor(f"xout{i}", [core_slots, 128], bf, kind="Internal").ap()
        for i in range(2)
    ]
    xall = [
        nc.dram_tensor(f"xall{i}", [n_cores * core_slots, 128], bf,
                       kind="Internal", addr_space="Shared").ap()
        for i in range(2)
    ]
    # pooled per-graph sums: each core's [256,128] partial is reduce-scattered
    # so the tiny ExternalOutput holds 32 fully-summed graph rows per core
    gfull_d = nc.dram_tensor("gfull", [256, 128], f32, kind="Internal").ap()
    gpart_d = nc.dram_tensor("gpart", [256 // n_cores, 128], f32,
                             kind="Internal").ap()
    gsum_d = nc.dram_tensor("gsum", [256 // n_cores, 128], f32,
                            kind="ExternalOutput").ap()
    if debug_tabs:
        dtab_d = nc.dram_tensor("dtab", [128, slots], bf,
                                kind="ExternalOutput").ap()
        dloc_d = nc.dram_tensor("dloc", [128, core_slots], bf,
                                kind="ExternalOutput").ap()
        dxi_d = nc.dram_tensor("dxi", [128, CHUNK * TPB * 128], bf,
                               kind="ExternalOutput").ap()
        dlo_d = nc.dram_tensor("dlo", [128, CHUNK * TL * 128], bf,
                               kind="ExternalOutput").ap()
        dhi_d = nc.dram_tensor("dhi", [128, CHUNK * TH * 128], bf,
                               kind="ExternalOutput").ap()
        dmsg_d = nc.dram_tensor("dmsg", [128, 128], bf,
                                kind="ExternalOutput").ap()
        dagg_d = nc.dram_tensor("dagg", [128, 128], f32,
                                kind="ExternalOutput").ap()

    rg = [list(range(n_cores))]

    with tile.TileContext(nc) as tc:
        # reassemble the shared blob from its 8 per-core slices
        # (collectives cannot read IO tensors -> stage to Internal first)
        nc.sync.dma_start(sstage_d, sblob_d)
        nc.gpsimd.collective_compute(
            "AllGather", mybir.AluOpType.bypass, replica_groups=rg,
            ins=[sstage_d], outs=[sall_d])

        with tc.tile_pool(name="persist", bufs=1) as persist:
            # persistent SBUF
            tab_s = persist.tile([128, ranks, 128], bf)
            loc_a = persist.tile([128, nblk, 128], bf, tag="loca")
            loc_b = persist.tile([128, nblk, 128], bf, tag="locb")
            loc_tabs = [loc_a, loc_b]
            dst_s = persist.tile([128, NT], f32)
            iota_s = persist.tile([128, 128], bf)
            io256_s = persist.tile([128, 256], bf)
            wxi_s = persist.tile([128, N_CONV, 256], bf)
            wxj_s = persist.tile([128, N_CONV, 256], bf)
            wea_s = persist.tile([101, N_CONV, 256], bf)
            g_s = persist.tile([128, N_CONV, 128], f32)
            b_s = persist.tile([128, N_CONV, 128], f32)
            gid_s = persist.tile([128, nblk], f32)
            eps_s = persist.tile([128, 1], f32)
            ones_s = persist.tile([128, 1], f32)

            nc.sync.dma_start(wxi_s[:], wxi_d)
            nc.sync.dma_start(wxj_s[:], wxj_d)
            nc.sync.dma_start(wea_s[:], wea_d)
            nc.sync.dma_start(gid_s[:], gid_d)
            nc.vector.memset(eps_s[:], 1e-5)
            nc.vector.memset(ones_s[:], 1.0)

            # ---------------- init: expansions + tables + edge features --
            with (
                tc.tile_pool(name="initp", bufs=1) as initp,
                tc.tile_pool(name="inits", bufs=3) as inits,
                tc.tile_pool(name="initps", bufs=2, space="PSUM") as initps,
            ):
                noffs_s = initp.tile([101, 1], f32, tag="noffs")
                cfs_s = initp.tile([101, 1], f32, tag="cfs")
                lnr_s = initp.tile([1, N_CONV * 256], f32, tag="lnr")
                ior_s = initp.tile([1, 256], f32, tag="ior")
                one1_s = initp.tile([1, 128], f32, tag="one1")
                on101_s = initp.tile([1, 101], f32, tag="on101")
                dstb_s = initp.tile([128, NT], mybir.dt.int8, tag="dstb")
                nc.sync.dma_start(noffs_s[:], noffs_d)
                nc.sync.dma_start(cfs_s[:], cfs_d)
                nc.sync.dma_start(lnr_s[:], lnr_d)
                nc.sync.dma_start(ior_s[:], ior_d)
                nc.vector.memset(one1_s[:], 1.0)
                nc.vector.memset(on101_s[:], 1.0)
                nc.sync.dma_start(dstb_s[:], dst_d)
                nc.vector.tensor_scalar(
                    out=dst_s[:], in0=dstb_s[:], scalar1=1.0, scalar2=None,
                    op0=mybir.AluOpType.mult)

                # replicate [16, W] index arrays into [128, W] DRAM via SBUF
                # (ixi ships as uint8 and is widened to i16 on the way)
                STW = 2048
                for src_d, dst_x, w, is_u8 in (
                    (ixi_d, ixi_x, S // 16, True),
                    (ixlo_d, ixlo_x, SLO // 16, False),
                    (ixhi_d, ixhi_x, SHI // 16, False),
                    (zall_d, zall_x, slots // 16, False),
                    (zown_d, zown_x, core_slots // 16, False),
                ):
                    for o in range(0, w, STW):
                        ww = min(STW, w - o)
                        if is_u8:
                            st8 = inits.tile([16, STW], mybir.dt.uint8,
                                             tag="ix_st8")
                            nc.sync.dma_start(st8[:, :ww], src_d[:, o:o + ww])
                            st = inits.tile([16, STW], i16, tag="ix_st")
                            nc.vector.tensor_scalar(
                                out=st[:, :ww], in0=st8[:, :ww], scalar1=0,
                                scalar2=None, op0=mybir.AluOpType.add)
                        else:
                            st = inits.tile([16, STW], i16, tag="ix_st")
                            nc.sync.dma_start(st[:, :ww], src_d[:, o:o + ww])
                        for k in range(8):
                            nc.sync.dma_start(
                                dst_x[k * 16:(k + 1) * 16, o:o + ww],
                                st[:, :ww])

                # iota / iota256 built by broadcasting a row over partitions
                pio = initps.tile([128, 512], f32, tag="pio")
                nc.tensor.matmul(pio[:, :256], one1_s[:], ior_s[:],
                                 start=True, stop=True)
                nc.scalar.activation(io256_s[:], pio[:, :256], AF.Copy)
                nc.scalar.activation(iota_s[:], pio[:, :128], AF.Copy)
                # LN gamma/beta broadcast
                for l in range(N_CONV):
                    pln = initps.tile([128, 512], f32, tag="pln")
                    nc.tensor.matmul(pln[:, :256], one1_s[:],
                                     lnr_s[:, l * 256:(l + 1) * 256],
                                     start=True, stop=True)
                    nc.scalar.activation(g_s[:, l, :], pln[:, :128], AF.Copy)
                    nc.scalar.activation(b_s[:, l, :], pln[:, 128:256], AF.Copy)

                # z-indexed gathers build the node tables from the embedding
                zallt = initp.tile([128, slots // 16], i16, tag="zall")
                nc.sync.dma_start(zallt[:], zall_x)
                zownt = initp.tile([128, core_slots // 16], i16, tag="zown")
                nc.sync.dma_start(zownt[:], zown_x)
                PIECE = 4096   # HW limit: big single gathers crash the device
                for o in range(0, slots, PIECE):
                    n = min(PIECE, slots - o)
                    nc.gpsimd.dma_gather(
                        tab_s[:, o // 128:(o + n) // 128, :], ewb_d,
                        zallt[:, o // 16:(o + n) // 16], n, n, 128,
                        transpose=False, single_packet=False)
                x0_t = initp.tile([128, nblk, 128], f32, tag="x0")
                for o in range(0, core_slots, PIECE):
                    n = min(PIECE, core_slots - o)
                    nc.gpsimd.dma_gather(
                        loc_a[:, o // 128:(o + n) // 128, :], ewb_d,
                        zownt[:, o // 16:(o + n) // 16], n, n, 128,
                        transpose=False, single_packet=False)
                    nc.gpsimd.dma_gather(
                        x0_t[:, o // 128:(o + n) // 128, :], ewf_d,
                        zownt[:, o // 16:(o + n) // 16], n, n, 128,
                        transpose=False, single_packet=False)
                nc.sync.dma_start(xinit.rearrange("(r p) c -> p r c", p=128),
                                  x0_t[:])

                # gaussian smearing: ea_k = exp(cfs_k * (d - offs_k)^2);
                # row 100 has cfs=0 -> exp(0)=1 (the conv-bias row).
                # d is broadcast over 101 partitions by a K=1 matmul.
                P = 512
                for off in range(0, S, P):
                    w = min(P, S - off)
                    uti = inits.tile([1, P], i16, tag="uti")
                    nc.sync.dma_start(uti[:, :w], u_d[:, off:off + w])
                    ut = inits.tile([1, P], f32, tag="ut")
                    nc.vector.tensor_scalar(
                        out=ut[:, :w], in0=uti[:, :w],
                        scalar1=float(UQMAX / 32767.0), scalar2=None,
                        op0=mybir.AluOpType.mult)
                    pe_ = initps.tile([101, P], f32, tag="pea")
                    nc.tensor.matmul(pe_[:, :w], on101_s[:], ut[:, :w],
                                     start=True, stop=True)
                    sq = inits.tile([101, P], f32, tag="sq")
                    nc.scalar.activation(sq[:, :w], pe_[:, :w], AF.Square,
                                         bias=noffs_s[:])
                    et = inits.tile([101, P], bf, tag="et")
                    nc.scalar.activation(et[:, :w], sq[:, :w], AF.Exp,
                                         scale=cfs_s[:])
                    nc.sync.dma_start(ea_h[:, off:off + w], et[:, :w])

            if debug_tabs:
                nc.sync.dma_start(dtab_d, tab_s.rearrange("p r c -> p (r c)"))
                nc.sync.dma_start(dloc_d, loc_a.rearrange("p r c -> p (r c)"))

            # ---------------- main conv layers ----------------------------
            with (
                tc.tile_pool(name="gxi", bufs=2) as gxi_p,
                tc.tile_pool(name="glo", bufs=2) as glo_p,
                tc.tile_pool(name="ghi", bufs=2) as ghi_p,
                tc.tile_pool(name="eat", bufs=2) as ea_p,
                tc.tile_pool(name="idx", bufs=2) as idx_p,
                tc.tile_pool(name="small", bufs=3) as small_p,
                tc.tile_pool(name="xio", bufs=2) as xio_p,
                tc.tile_pool(name="stats", bufs=2) as stats_p,
                tc.tile_pool(name="zc", bufs=3, space="PSUM") as zc_p,
                tc.tile_pool(name="agg", bufs=3, space="PSUM") as agg_p,
                tc.tile_pool(name="gsm", bufs=1, space="PSUM") as gsm_p,
            ):
                n_chunks = (nblk + CHUNK - 1) // CHUNK
                tab_flat = tab_s.rearrange("p r c -> p (r c)")
                tab_lo_view = tab_flat[:, : lo_ranks * 128]
                tab_hi_view = tab_flat[:, lo_ranks * 128:]
                gsum0 = gsm_p.tile([128, 128], f32, tag="g0")
                gsum1 = gsm_p.tile([128, 128], f32, tag="g1")

                for layer in range(N_CONV):
                    last = layer == N_CONV - 1
                    xold_src = xinit if layer == 0 else xmast[layer - 1]
                    loc_read = loc_tabs[layer % 2]
                    loc_write = loc_tabs[(layer + 1) % 2]
                    loc_flat = loc_read.rearrange("p r c -> p (r c)")

                    for ch in range(n_chunks):
                        b0 = ch * CHUNK
                        nb = min(CHUNK, nblk - b0)  # blocks in this chunk
                        n_ti = nb * TPB             # xi tiles in chunk
                        n_tl = nb * TL
                        n_th = nb * TH

                        # ---- per-chunk loads -------------------------------
                        ixi_t = idx_p.tile([128, CHUNK * TPB * 8], i16, tag="ixi")
                        ixlo_t = idx_p.tile([128, CHUNK * TL * 8], i16, tag="ixlo")
                        ixhi_t = idx_p.tile([128, CHUNK * TH * 8], i16, tag="ixhi")
                        ea_t = ea_p.tile([101, CHUNK * TPB * 128], bf, tag="ea")
                        c0 = b0 * TPB * 8
                        nc.sync.dma_start(ixi_t[:, :n_ti * 8],
                                          ixi_x[:, c0:c0 + n_ti * 8])
                        nc.sync.dma_start(
                            ixlo_t[:, :n_tl * 8],
                            ixlo_x[:, b0 * TL * 8: b0 * TL * 8 + n_tl * 8])
                        nc.sync.dma_start(
                            ixhi_t[:, :n_th * 8],
                            ixhi_x[:, b0 * TH * 8: b0 * TH * 8 + n_th * 8])
                        nc.sync.dma_start(
                            ea_t[:, :n_ti * 128],
                            ea_h[:, b0 * TPB * 128: (b0 * TPB + n_ti) * 128])

                        # ---- gathers (SBUF-source, transposed, bf16) -------
                        xi_g = gxi_p.tile([128, 1, CHUNK * TPB * 128], bf, tag="xi")
                        lo_g = glo_p.tile([128, 1, CHUNK * TL * 128], bf, tag="lo")
                        hi_g = ghi_p.tile([128, 1, CHUNK * TH * 128], bf, tag="hi")
                        loc_view = loc_flat[:, b0 * 128:(b0 + nb) * 128]
                        nc.gpsimd.dma_gather(
                            xi_g[:, :, :n_ti * 128], loc_view, ixi_t[:, :n_ti * 8],
                            n_ti * 128, n_ti * 128, 128,
                            transpose=True, sbuf_tokens_per_rank=128,
                            sbuf_free_dim_per_rank=256, single_packet=False)
                        nc.gpsimd.dma_gather(
                            lo_g[:, :, :n_tl * 128], tab_lo_view, ixlo_t[:, :n_tl * 8],
                            n_tl * 128, n_tl * 128, 128,
                            transpose=True, sbuf_tokens_per_rank=128,
                            sbuf_free_dim_per_rank=256, single_packet=False)
                        nc.gpsimd.dma_gather(
                            hi_g[:, :, :n_th * 128], tab_hi_view, ixhi_t[:, :n_th * 8],
                            n_th * 128, n_th * 128, 128,
                            transpose=True, sbuf_tokens_per_rank=128,
                            sbuf_free_dim_per_rank=256, single_packet=False)

                        if debug_tabs and layer == 0 and ch == 0:
                            nc.sync.dma_start(dxi_d, xi_g[:, 0, :])
                            nc.sync.dma_start(dlo_d, lo_g[:, 0, :])
                            nc.sync.dma_start(dhi_d, hi_g[:, 0, :])

                        # ---- per-block compute -----------------------------
                        for bi in range(nb):
                            blk = b0 + bi
                            agg = agg_p.tile([128, 128], f32, tag="agg")
                            for t in range(TPB):
                                is_lo = t < TL
                                xi_sl = xi_g[:, 0, (bi * TPB + t) * 128:
                                             (bi * TPB + t + 1) * 128]
                                if is_lo:
                                    xj_sl = lo_g[:, 0, (bi * TL + t) * 128:
                                                 (bi * TL + t + 1) * 128]
                                else:
                                    th = t - TL
                                    xj_sl = hi_g[:, 0, (bi * TH + th) * 128:
                                                 (bi * TH + th + 1) * 128]
                                ea_sl = ea_t[:, (bi * TPB + t) * 128:
                                             (bi * TPB + t + 1) * 128]

                                zc = zc_p.tile([128, 256], f32, tag="zc")
                                nc.tensor.matmul(zc[:], xi_sl, wxi_s[:, layer, :],
                                                 start=True, stop=False)
                                nc.tensor.matmul(zc[:], xj_sl, wxj_s[:, layer, :],
                                                 start=False, stop=False)
                                nc.tensor.matmul(zc[:], ea_sl, wea_s[:, layer, :],
                                                 start=False, stop=True)

                                sel = small_p.tile([128, 128], bf, tag="sel")
                                nc.vector.tensor_scalar(
                                    out=sel[:], in0=iota_s[:],
                                    scalar1=dst_s[:, blk * TPB + t: blk * TPB + t + 1],
                                    scalar2=None, op0=mybir.AluOpType.is_equal)

                                # zc holds [-z1 | z2] (z1-half weights
                                # sign-flipped on host).
                                # msg = softplus(z2) * sigmoid(z1)
                                #     = ln(1+e^{z2}) / (1 + e^{-z1})
                                ez = small_p.tile([128, 256], f32, tag="ez")
                                nc.scalar.activation(ez[:], zc[:], AF.Exp)
                                sp = small_p.tile([128, 128], bf, tag="sp")
                                nc.scalar.activation(sp[:], ez[:, 128:256],
                                                     AF.Ln, bias=ones_s[:])
                                u1 = small_p.tile([128, 128], f32, tag="u1")
                                nc.vector.tensor_scalar(
                                    out=u1[:], in0=ez[:, 0:128], scalar1=1.0,
                                    scalar2=None, op0=mybir.AluOpType.add)
                                rcp = small_p.tile([128, 128], f32, tag="rcp")
                                nc.vector.reciprocal(rcp[:], u1[:])
                                msg = small_p.tile([128, 128], bf, tag="msg")
                                nc.vector.tensor_mul(msg[:], sp[:], rcp[:])

                                nc.tensor.matmul(agg[:], sel[:], msg[:],
                                                 start=(t == 0), stop=(t == TPB - 1))
                                if debug_tabs and layer == 0 and blk == 0 and t == 0:
                                    nc.sync.dma_start(dmsg_d, msg[:])

                            # ---- block epilogue: LN + residual + softplus --
                            if debug_tabs and layer == 0 and blk == 0:
                                dag = xio_p.tile([128, 128], f32, tag="dag")
                                nc.scalar.activation(dag[:], agg[:], AF.Copy)
                                nc.sync.dma_start(dagg_d, dag[:])
                            xold = xio_p.tile([128, 128], f32, tag="xold")
                            nc.sync.dma_start(
                                xold[:], xold_src[blk * 128:(blk + 1) * 128, :])

                            st = stats_p.tile([128, 6], f32, tag="bn")
                            nc.vector.bn_stats(out=st[:], in_=agg[:])
                            mv = stats_p.tile([128, 2], f32, tag="mv")
                            nc.vector.bn_aggr(out=mv[:], in_=st[:])
                            # rstd = exp(-0.5 * ln(var + eps))
                            lnv = stats_p.tile([128, 1], f32, tag="lnv")
                            nc.scalar.activation(lnv[:], mv[:, 1:2], AF.Ln,
                                                 bias=eps_s[:])
                            rstd = stats_p.tile([128, 1], f32, tag="rstd")
                            nc.scalar.activation(rstd[:], lnv[:], AF.Exp,
                                                 scale=-0.5)

                            xn = xio_p.tile([128, 128], f32, tag="xn")
                            nc.vector.tensor_scalar(
                                out=xn[:], in0=agg[:], scalar1=mv[:, 0:1],
                                scalar2=rstd[:], op0=mybir.AluOpType.subtract,
                                op1=mybir.AluOpType.mult)
                            nc.vector.tensor_mul(xn[:], xn[:], g_s[:, layer, :])
                            nc.vector.tensor_add(xn[:], xn[:], b_s[:, layer, :])
                            nc.vector.tensor_add(xn[:], xn[:], xold[:])

                            # softplus(xn) = ln(1 + e^{xn})
                            exn = xio_p.tile([128, 128], f32, tag="exn")
                            nc.scalar.activation(exn[:], xn[:], AF.Exp)
                            xnew = xio_p.tile([128, 128], f32, tag="xnew")
                            nc.scalar.activation(xnew[:], exn[:], AF.Ln,
                                                 bias=ones_s[:])
                            if not last:
                                # bf16 copy into next layer's local gather table
                                nc.scalar.activation(loc_write[:, blk, :],
                                                     xnew[:], AF.Copy)
                                nc.sync.dma_start(
                                    xmast[layer][blk * 128:(blk + 1) * 128, :],
                                    xnew[:])
                            else:
                                # pooled per-graph sums: gsum[g,:] += x[slot,:]
                                gsel = xio_p.tile([128, 256], f32, tag="gsel")
                                nc.vector.tensor_scalar(
                                    out=gsel[:], in0=io256_s[:],
                                    scalar1=gid_s[:, blk:blk + 1],
                                    scalar2=None, op0=mybir.AluOpType.is_equal)
                                nc.tensor.matmul(gsum0[:], gsel[:, 0:128],
                                                 xnew[:], start=(blk == 0),
                                                 stop=(blk == nblk - 1))
                                nc.tensor.matmul(gsum1[:], gsel[:, 128:256],
                                                 xnew[:], start=(blk == 0),
                                                 stop=(blk == nblk - 1))

                    # ---- exchange (layers 0,1): slice -> AllGather -> table
                    if not last:
                        nc.sync.dma_start(
                            xout[layer].rearrange("(r p) c -> p r c", p=128),
                            loc_write[:])
                        nc.gpsimd.collective_compute(
                            "AllGather", mybir.AluOpType.bypass,
                            replica_groups=rg,
                            ins=[xout[layer][:]], outs=[xall[layer][:]])
                        nc.sync.dma_start(
                            tab_s[:],
                            xall[layer].rearrange("(r p) c -> p r c", p=128))

                # ---- write pooled output ---------------------------------
                gsb = xio_p.tile([128, 256], f32, tag="gsb")
                nc.scalar.activation(gsb[:, 0:128], gsum0[:], AF.Copy)
                nc.scalar.activation(gsb[:, 128:256], gsum1[:], AF.Copy)
                nc.sync.dma_start(gfull_d[0:128, :], gsb[:, 0:128])
                nc.sync.dma_start(gfull_d[128:256, :], gsb[:, 128:256])
                nc.gpsimd.collective_compute(
                    "ReduceScatter", mybir.AluOpType.add, replica_groups=rg,
                    ins=[gfull_d], outs=[gpart_d])
                nc.sync.dma_start(gsum_d, gpart_d)

    nc.compile()
    return nc


# --------------------------------------------------------------------------
# Host preprocessing
# --------------------------------------------------------------------------

def _softplus(x):
    return np.log1p(np.exp(-np.abs(x))) + np.maximum(x, 0.0)


def _snake_slots(n, n_bins):
    """Slot offsets (bin*128 + round) for n items dealt snake-wise, in the
    order of the sorted item list."""
    idx = np.arange(n)
    r = idx // n_bins
    k = idx % n_bins
    bins = np.where(r % 2 == 0, k, n_bins - 1 - k)
    return bins * 128 + r


def _wrap16(arr):
    # [n] int16 -> [16, n/16], idx i at (i%16, i//16)
    return np.ascontiguousarray(arr.reshape(-1, 16).T)


def preprocess(z, R, edge_index, batch, embedding, emb_w, emb_b, conv_w, conv_b,
               ln_g, ln_b, n_nodes=N_NODES, n_cores=N_CORES, nblk=NBLK,
               lo_slots=LO_SLOTS, edge_d=EDGE_D, cutoff=CUTOFF):
    blocks = n_cores * nblk
    slots = blocks * 128
    core_slots = nblk * 128
    lo_blocks = lo_slots // 128
    n_edges = edge_index.shape[1]
    src = np.asarray(edge_index[0], np.int64)
    dst = np.asarray(edge_index[1], np.int64)

    # edge distances on host (smearing runs on device)
    Rf = np.asarray(R, np.float32)
    d = np.linalg.norm(Rf[src] - Rf[dst], axis=-1)  # [E] f32

    # node permutation: balance per-block in-degrees; L = orig nodes < lo_slots
    islo_e = src < lo_slots
    a = np.bincount(dst[islo_e], minlength=n_nodes)
    b = np.bincount(dst[~islo_e], minlength=n_nodes)
    w = a + b
    # L-nodes -> slots [0, lo_slots); rest -> [lo_slots, slots)
    ordL = np.argsort(-w[:lo_slots], kind="stable")
    ordH = np.argsort(-w[lo_slots:], kind="stable") + lo_slots
    perm = np.full(n_nodes, -1, np.int64)
    perm[ordL] = _snake_slots(ordL.size, lo_blocks)
    perm[ordH] = _snake_slots(ordH.size, blocks - lo_blocks) + lo_slots
    assert perm.min() >= 0

    es, ed = perm[src], perm[dst]
    blk = ed // 128

    lo_cnt = np.bincount(blk[islo_e], minlength=blocks)
    hi_cnt = np.bincount(blk[~islo_e], minlength=blocks)
    TL = int(-(-lo_cnt.max() // 128))
    TH = int(-(-hi_cnt.max() // 128))
    TPB = TL + TH
    S = nblk * TPB * 128

    # edge slot assignment: within block, lows first then highs
    key = blk * 2 + (~islo_e).astype(np.int64)
    eorder = np.argsort(key, kind="stable")
    ks = key[eorder]
    # position within each (block, half) run
    runstart = np.r_[0, np.flatnonzero(np.diff(ks)) + 1]
    runid = np.zeros(n_edges, np.int64)
    runid[runstart[1:]] = 1
    runid = np.cumsum(runid)
    pos = np.arange(n_edges) - runstart[runid]
    eb = ks // 2
    ehalf = ks % 2
    base = eb * TPB * 128 + ehalf * (TL * 128)
    eslot_g = base + pos                       # global edge slot (per full graph)
    # per-core arrays
    core_of = eb // nblk
    eslot = eslot_g - core_of * (nblk * TPB * 128)

    ixi = np.zeros((n_cores, S), np.int16)
    ixlo = np.zeros((n_cores, nblk * TL * 128), np.int16)
    ixhi = np.zeros((n_cores, nblk * TH * 128), np.int16)
    dstv = np.full((n_cores, nblk * TPB, 128), -1.0, np.float32)
    u = np.zeros((n_cores, 1, S), np.float32)

    e_src = es[eorder]
    e_dst = ed[eorder]
    e_lo = ehalf == 0
    d_o = d[eorder]

    for c in range(n_cores):
        m = core_of == c
        sl = eslot[m]
        # xi: dst local to the chunk's 2-block view (fits uint8)
        dloc = (e_dst[m] - c * core_slots) % (CHUNK * 128)
        ixi[c][sl] = dloc.astype(np.int16)
        assert dloc.max(initial=0) < 256
        # xj
        mlo = m & e_lo
        mhi = m & ~e_lo
        slo_ = eslot[mlo]
        bb = slo_ // (TPB * 128)
        off = slo_ - bb * (TPB * 128)
        ixlo[c][bb * TL * 128 + off] = e_src[mlo].astype(np.int16)
        shi_ = eslot[mhi]
        bb = shi_ // (TPB * 128)
        off = shi_ - bb * (TPB * 128) - TL * 128
        ixhi[c][bb * TH * 128 + off] = (e_src[mhi] - lo_slots).astype(np.int16)
        # dst one-hot value, edge distances
        dstv[c].reshape(-1)[sl] = (e_dst[m] % 128).astype(np.float32)
        u[c, 0, sl] = d_o[m].astype(np.float32)

    # z tables (slot -> atom type; empty slots -> 100 which maps to a 0 row)
    zslot = np.full(slots, 100, np.int16)
    zslot[perm] = np.asarray(z, np.int16)
    # graph-id per slot (empty -> -1, excluded from pooling)
    gslot = np.full(slots, -1.0, np.float32)
    gslot[perm] = np.asarray(batch, np.float32)

    # embedding rows
    EWf = np.zeros((128, 128), np.float32)
    EWf[:100] = (np.asarray(embedding, np.float32)
                 @ np.asarray(emb_w, np.float32)
                 + np.asarray(emb_b, np.float32))
    EWb = EWf.astype(BF16)

    # conv weights; z1-half output columns sign-flipped so the device computes
    # [-z1 | z2] and can use exp/ln-only activations (one act table)
    cw = np.asarray(conv_w, np.float32).copy()
    cb = np.asarray(conv_b, np.float32).copy()
    cw[:, :, :128] *= -1.0
    cb[:, :128] *= -1.0
    wxi = np.ascontiguousarray(cw[:, :128, :].transpose(1, 0, 2)).astype(BF16)
    wxj = np.ascontiguousarray(cw[:, 128:256, :].transpose(1, 0, 2)).astype(BF16)
    wea = np.concatenate([cw[:, 256:, :], cb[:, None, :]], axis=1)
    wea = np.ascontiguousarray(wea.transpose(1, 0, 2)).astype(BF16)

    # LN gamma/beta rows
    lnr = np.concatenate(
        [np.concatenate([np.asarray(ln_g, np.float32)[l],
                         np.asarray(ln_b, np.float32)[l]])
         for l in range(cw.shape[0])])[None, :]

    # smearing: ea_k = exp(cfs_k * (d - offs_k)^2); cfs[100]=0 -> bias row 1
    offs = np.linspace(0.0, cutoff, edge_d, dtype=np.float32)
    coeff = np.float32(-0.5 / (offs[1] - offs[0]) ** 2)
    noffs = np.zeros((101, 1), np.float32)
    noffs[:edge_d, 0] = -offs
    cfs = np.zeros((101, 1), np.float32)
    cfs[:edge_d, 0] = coeff

    ior = np.arange(256, dtype=np.float32)[None, :]

    # ---- pack blobs ------------------------------------------------------
    uents, sents, BU, BS, BS8 = _layout(TL, TH, nblk, ranks=blocks,
                                        n_cores=n_cores)

    def pack(ents, arrays, nbytes):
        blob = np.zeros(nbytes // 2, np.int16)
        bv = blob.view(np.uint8)
        for name, (off, shape, dt_) in ents.items():
            a = np.ascontiguousarray(arrays[name])
            assert a.shape == tuple(shape) and a.dtype == np.dtype(dt_), \
                (name, a.shape, shape, a.dtype, dt_)
            bv[off:off + a.nbytes] = a.view(np.uint8).ravel()
        return blob

    sblob = pack(sents, {
        "zall": _wrap16(zslot), "ewb": EWb, "ewf": EWf,
        "wxi": wxi, "wxj": wxj, "wea": wea,
        "lnr": lnr, "ior": ior, "noffs": noffs, "cfs": cfs,
    }, BS)

    in_maps = []
    for c in range(n_cores):
        sl0 = c * core_slots
        uq = np.round(np.minimum(u[c], UQMAX) * (32767.0 / UQMAX)
                      ).astype(np.int16)
        ublob = pack(uents, {
            "u": uq,
            "ixi": _wrap16(ixi[c]).astype(np.uint8),
            "ixlo": _wrap16(ixlo[c]),
            "ixhi": _wrap16(ixhi[c]),
            "zown": _wrap16(zslot[sl0:sl0 + core_slots]),
            "dstv": np.ascontiguousarray(
                dstv[c].transpose(1, 0)).astype(np.int8),
            "gid": np.ascontiguousarray(
                gslot[sl0:sl0 + core_slots].reshape(nblk, 128).T),
        }, BU)
        in_maps.append({
            "ublob": ublob,
            "sblob": sblob[c * BS8 // 2:(c + 1) * BS8 // 2],
        })
    return in_maps, TL, TH


# --------------------------------------------------------------------------
# execution: cached jitted SPMD runner (PJRT via bass2jax custom call)
# --------------------------------------------------------------------------

class _Results:
    """Minimal stand-in for BassKernelResults (test.py reads exec_time_ns)."""

    def __init__(self, results):
        self.results = results
        self.exec_time_ns = None


class _Runner:
    """Compile once, then run full numpy in_maps -> numpy outputs.

    Reimplements bass_utils.run_bass_kernel_spmd's axon path with a CACHED
    jitted callable: a fresh jax.jit per call costs ~2.5 s of re-trace/
    re-lowering for an identical program. Each run() still performs the full
    host->device transfer of every input, the NEFF execution on all 8 cores,
    and the device->host readback of the outputs.
    """

    def __init__(self, nc, n_cores):
        import jax
        from jax.sharding import Mesh, PartitionSpec
        from jax.experimental.shard_map import shard_map
        from concourse import bass2jax

        bass2jax.install_neuronx_cc_hook()
        self.n_cores = n_cores
        partition_name = (nc.partition_id_tensor.name
                          if nc.partition_id_tensor else None)
        in_names, out_names, out_avals, zero_outs = [], [], [], []
        for alloc in nc.m.functions[0].allocations:
            if not isinstance(alloc, mybir.MemoryLocationSet):
                continue
            name = alloc.memorylocations[0].name
            if alloc.kind == "ExternalInput":
                if name != partition_name:
                    in_names.append(name)
            elif alloc.kind == "ExternalOutput":
                shape = tuple(alloc.tensor_shape)
                dtype = mybir.dt.np(alloc.dtype)
                out_names.append(name)
                out_avals.append(jax.core.ShapedArray(shape, dtype))
                zero_outs.append(np.zeros((n_cores * shape[0], *shape[1:]),
                                          dtype))
        self.in_names = in_names
        self.out_names = out_names
        self.out_shapes = [tuple(a.shape) for a in out_avals]
        self.zero_outs = zero_outs
        n_params = len(in_names)
        all_in = in_names + out_names + (
            [partition_name] if partition_name else [])

        def _body(*args):
            operands = list(args)
            if partition_name is not None:
                operands.append(bass2jax.partition_id_tensor())
            outs = bass2jax._bass_exec_p.bind(
                *operands, out_avals=tuple(out_avals),
                in_names=tuple(all_in), out_names=tuple(out_names),
                lowering_input_output_aliases=(),
                sim_require_finite=True, sim_require_nnan=True, nc=nc)
            return tuple(outs)

        devs = jax.devices()[:n_cores]
        assert len(devs) == n_cores
        mesh = Mesh(np.asarray(devs), ("core",))
        n_outs = len(out_avals)
        self._fn = jax.jit(
            shard_map(_body, mesh=mesh,
                      in_specs=(PartitionSpec("core"),) * (n_params + n_outs),
                      out_specs=(PartitionSpec("core"),) * n_outs,
                      check_rep=False),
            donate_argnums=tuple(range(n_params, n_params + n_outs)),
            keep_unused=True)
    def run(self, in_maps):
        concat_in = [
            np.concatenate([np.asarray(m[n]) for m in in_maps], axis=0)
            for n in self.in_names]
        outs = self._fn(*concat_in, *self.zero_outs)
        n = self.n_cores
        return _Results([
            {name: np.asarray(outs[i]).reshape(n, *self.out_shapes[i])[c]
             for i, name in enumerate(self.out_names)}
            for c in range(n)])


def kernel(z, R, edge_index, batch, embedding, emb_w, emb_b, conv_w, conv_b,
           ln_g, ln_b, cfc_w, cfc_b, fc_w, fc_b, out_w, out_b):
    in_maps, TL, TH = preprocess(
        z, R, edge_index, batch, embedding, emb_w, emb_b, conv_w, conv_b,
        ln_g, ln_b)

    key = (TL, TH)
    if key not in _NC_CACHE:
        nc = build_nc(TL, TH)
        _NC_CACHE[key] = _Runner(nc, N_CORES)
    runner = _NC_CACHE[key]

    res = runner.run(in_maps)
    global LAST_RESULTS, LAST_RERUN_S
    LAST_RESULTS = res
    if _os.environ.get("KERNEL_RERUN", "1") != "0":
        import time as _time
        t0 = _time.time()
        runner.run(in_maps)
        LAST_RERUN_S = _time.time() - t0

    gs = np.concatenate([res.results[c]["gsum"] for c in range(N_CORES)],
                        axis=0)  # [256, 128] fully-summed (reduce-scattered)

    batch = np.asarray(batch, np.int64)
    cnts = np.bincount(batch, minlength=N_GRAPHS).astype(np.float32)
    mol = gs / np.maximum(cnts, 1.0)[:, None]

    h = _softplus(mol @ np.asarray(cfc_w, np.float32) + np.asarray(cfc_b, np.float32))
    for l in range(np.asarray(fc_w).shape[0]):
        h = _softplus(h @ np.asarray(fc_w[l], np.float32)
                      + np.asarray(fc_b[l], np.float32))
    out = h @ np.asarray(out_w, np.float32) + np.asarray(out_b, np.float32)
    return out.astype(np.float32)



# revision 4
# speedup vs baseline: 31.7788x; 31.7788x over previous
"""CGCNN (no BN) message-passing GNN on 8 Trainium2 NeuronCores.

Strategy (self-contained; shapes hardcoded from the problem spec):
 - Nodes are permuted on the host into 392 blocks of 128 slots, balancing
   per-block in-edge counts. Cores own 49 contiguous blocks (6272 slots).
 - Edges are owned by the core that owns their destination block; within a
   block, edges are split by source-slot half (<32768 vs >=32768) so each
   128-edge tile gathers from a single int16-indexable table view, then
   padded to a uniform (TL, TH) tile count per block so all 8 cores run one
   SPMD program.
 - Host->device traffic is minimized (the axon relay is ~30 MB/s):
     * edge features (gaussian smearing) are computed ON DEVICE from a
       [2, S] f32 (d^2, d) array via a K=2 matmul + Exp, cached in DRAM;
     * the full bf16 gather table AND the f32 x0 residual table are built
       ON DEVICE by dma_gather of embedding rows indexed by z;
     * gather index arrays are shipped once, un-replicated ([16, n/16]) and
       expanded to the 128-partition layout on device;
     * LN params / iota matrices are built on device from row vectors;
     * per-graph mean-pooling runs on device (one-hot matmuls accumulated
       in PSUM over the last conv layer), so each core returns only a
       [256, 128] f32 partial sum.
 - Per 128-edge tile on device: dma_gather (SBUF source, transposed) pulls
   x[src] / x[dst] columns in channel-major bf16; three PE matmuls
   (xi@W1 + xj@W2 + eaT@W3b) accumulate the conv pre-activation in PSUM;
   ACT computes sigmoid/softplus; DVE builds a one-hot dst matrix which PE
   uses to segment-sum messages into the block accumulator.
 - LayerNorm + residual + softplus per block in f32; updated x is written to
   a bf16 local table (for x[dst] gathers) and, between conv layers, an
   in-kernel AllGather replicates every core's slice into the full bf16
   gather table.
 - The tiny pooled-MLP head runs on the host in f32.
"""

import os as _os
import numpy as np
import ml_dtypes

import concourse.bass as bass
import concourse.tile as tile
from concourse import bacc, mybir

BF16 = ml_dtypes.bfloat16

# Problem constants
N_NODES, N_EDGES, NODE_D, EDGE_D, EMB_D, N_GRAPHS = 50000, 800000, 128, 100, 92, 256
N_CONV, FC_D, N_FC, CUTOFF = 3, 128, 2, 6.0

LAST_RESULTS = None        # BassKernelResults of the most recent run (for tests)
LAST_RERUN_S = None        # wall seconds of a warm re-execution
LAST_RUN = None            # (runner, in_maps) of the most recent run (for tests)

N_CORES = 8
UQMAX = 6.625              # d quantization range (beyond it all gaussians ~ 0)
SLOTS = 50176              # 392 blocks * 128
BLOCKS = SLOTS // 128      # 392
NBLK = BLOCKS // N_CORES   # 49 blocks per core
CORE_SLOTS = NBLK * 128    # 6272
LO_SLOTS = 32768           # slots gatherable from the low table view
CHUNK = 2                  # blocks processed per gather chunk

_NC_CACHE = {}


# --------------------------------------------------------------------------
# Input blob layout (shared between host packing and device program)
# --------------------------------------------------------------------------

def _layout(TL, TH, nblk=NBLK, ranks=BLOCKS, n_cores=N_CORES):
    """Byte layout of the per-core-unique and shared input blobs.

    Returns (uents, sents, BU, BS, BS8): entry dicts name -> (off, shape,
    np_dtype), blob sizes in bytes. All offsets 512-aligned; BS is a
    multiple of n_cores*512 so the shared blob splits evenly.
    """
    TPB = TL + TH
    NT = nblk * TPB
    S = NT * 128
    SLO = nblk * TL * 128
    SHI = nblk * TH * 128
    slots = ranks * 128
    core_slots = nblk * 128

    uspec = [
        ("u", (1, S), np.int16),        # d quantized to [0, UQMAX]
        ("ixi", (16, S // 16), np.uint8),
        ("ixlo", (16, SLO // 16), np.int16),
        ("ixhi", (16, SHI // 16), np.int16),
        ("zown", (16, core_slots // 16), np.int16),
        ("dstv", (128, NT), np.int8),
        ("gid", (128, nblk), np.float32),
    ]
    sspec = [
        ("zall", (16, slots // 16), np.int16),
        ("ewb", (128, 128), BF16),
        ("ewf", (128, 128), np.float32),
        ("wxi", (128, N_CONV, 256), BF16),
        ("wxj", (128, N_CONV, 256), BF16),
        ("wea", (101, N_CONV, 256), BF16),
        ("lnr", (1, N_CONV * 256), np.float32),
        ("ior", (1, 256), np.float32),
        ("noffs", (101, 1), np.float32),
        ("cfs", (101, 1), np.float32),
    ]

    def place(spec):
        ents, off = {}, 0
        for name, shape, dt_ in spec:
            nbytes = int(np.prod(shape)) * np.dtype(dt_).itemsize
            ents[name] = (off, shape, dt_)
            off += -(-nbytes // 512) * 512
        return ents, off

    uents, BU = place(uspec)
    sents, BS = place(sspec)
    BU = -(-BU // 512) * 512
    align = n_cores * 512
    BS = -(-BS // align) * align
    return uents, sents, BU, BS, BS // n_cores


_NP2MY = None


def _my_dt(np_dt):
    global _NP2MY
    if _NP2MY is None:
        _NP2MY = {np.dtype(np.float32): mybir.dt.float32,
                  np.dtype(np.int16): mybir.dt.int16,
                  np.dtype(np.int8): mybir.dt.int8,
                  np.dtype(np.uint8): mybir.dt.uint8,
                  np.dtype(BF16): mybir.dt.bfloat16}
    return _NP2MY[np.dtype(np_dt)]


# --------------------------------------------------------------------------
# Device program
# --------------------------------------------------------------------------

def build_nc(TL, TH, nblk=NBLK, ranks=BLOCKS, n_cores=N_CORES,
             lo_ranks=LO_SLOTS // 128, debug_tabs=False):
    """Build the SPMD Bass program. TL/TH = low/high tiles per block."""
    TPB = TL + TH                 # tiles per block
    NT = nblk * TPB               # tiles per core
    S = NT * 128                  # edge slots per core
    SLO = nblk * TL * 128
    SHI = nblk * TH * 128
    slots = ranks * 128
    core_slots = nblk * 128
    f32, bf, i16 = mybir.dt.float32, mybir.dt.bfloat16, mybir.dt.int16
    AF = mybir.ActivationFunctionType

    nc = bacc.Bacc("TRN2", target_bir_lowering=False, debug=False,
                   num_devices=n_cores)

    # ---- external inputs: two flat blobs ---------------------------------
    uents, sents, BU, BS, BS8 = _layout(TL, TH, nblk, ranks, n_cores)
    ublob_d = nc.dram_tensor("ublob", [BU // 2], i16, kind="ExternalInput").ap()
    sblob_d = nc.dram_tensor("sblob", [BS8 // 2], i16, kind="ExternalInput").ap()
    sstage_d = nc.dram_tensor("sstage", [BS8 // 2], i16, kind="Internal").ap()
    sall_d = nc.dram_tensor("sall", [BS // 2], i16, kind="Internal",
                            addr_space="Shared").ap()

    def V(ents, base):
        def view(name):
            off, shape, dt_ = ents[name]
            nbytes = int(np.prod(shape)) * np.dtype(dt_).itemsize
            v = base[off // 2: (off + nbytes) // 2].bitcast(_my_dt(dt_))
            if len(shape) == 2:
                v = v.rearrange("(a b) -> a b", b=shape[1])
            elif len(shape) == 3:
                v = v.rearrange("(a b c) -> a b c", b=shape[1], c=shape[2])
            return v
        return view

    UV = V(uents, ublob_d)
    SV = V(sents, sall_d)
    u_d = UV("u")
    ixi_d, ixlo_d, ixhi_d = UV("ixi"), UV("ixlo"), UV("ixhi")
    zown_d, dst_d, gid_d = UV("zown"), UV("dstv"), UV("gid")
    zall_d, ewb_d, ewf_d = SV("zall"), SV("ewb"), SV("ewf")
    wxi_d, wxj_d, wea_d = SV("wxi"), SV("wxj"), SV("wea")
    lnr_d, ior_d = SV("lnr"), SV("ior")
    noffs_d, cfs_d = SV("noffs"), SV("cfs")

    # ---- internal DRAM ---------------------------------------------------
    ea_h = nc.dram_tensor("ea_h", [101, S], bf, kind="Internal").ap()
    ixi_x = nc.dram_tensor("ixi_x", [128, S // 16], i16, kind="Internal").ap()
    ixlo_x = nc.dram_tensor("ixlo_x", [128, SLO // 16], i16, kind="Internal").ap()
    ixhi_x = nc.dram_tensor("ixhi_x", [128, SHI // 16], i16, kind="Internal").ap()
    zall_x = nc.dram_tensor("zall_x", [128, slots // 16], i16, kind="Internal").ap()
    zown_x = nc.dram_tensor("zown_x", [128, core_slots // 16], i16,
                            kind="Internal").ap()
    xinit = nc.dram_tensor("xinit", [core_slots, 128], f32, kind="Internal").ap()
    xmast = [
        nc.dram_tensor(f"xmast{i}", [core_slots, 128], f32, kind="Internal").ap()
        for i in range(2)
    ]
    xout = [
        nc.dram_tensor(f"xout{i}", [core_slots, 128], bf, kind="Internal").ap()
        for i in range(2)
    ]
    xall = [
        nc.dram_tensor(f"xall{i}", [n_cores * core_slots, 128], bf,
                       kind="Internal", addr_space="Shared").ap()
        for i in range(2)
    ]
    # pooled per-graph sums: each core's [256,128] partial is reduce-scattered
    # so the tiny ExternalOutput holds 32 fully-summed graph rows per core
    gfull_d = nc.dram_tensor("gfull", [256, 128], f32, kind="Internal").ap()
    gpart_d = nc.dram_tensor("gpart", [256 // n_cores, 128], f32,
                             kind="Internal").ap()
    gsum_d = nc.dram_tensor("gsum", [256 // n_cores, 128], f32,
                            kind="ExternalOutput").ap()
    if debug_tabs:
        dtab_d = nc.dram_tensor("dtab", [128, slots], bf,
                                kind="ExternalOutput").ap()
        dloc_d = nc.dram_tensor("dloc", [128, core_slots], bf,
                                kind="ExternalOutput").ap()
        dxi_d = nc.dram_tensor("dxi", [128, CHUNK * TPB * 128], bf,
                               kind="ExternalOutput").ap()
        dlo_d = nc.dram_tensor("dlo", [128, CHUNK * TL * 128], bf,
                               kind="ExternalOutput").ap()
        dhi_d = nc.dram_tensor("dhi", [128, CHUNK * TH * 128], bf,
                               kind="ExternalOutput").ap()
        dmsg_d = nc.dram_tensor("dmsg", [128, 128], bf,
                                kind="ExternalOutput").ap()
        dagg_d = nc.dram_tensor("dagg", [128, 128], f32,
                                kind="ExternalOutput").ap()

    rg = [list(range(n_cores))]

    with tile.TileContext(nc) as tc:
        # reassemble the shared blob from its 8 per-core slices
        # (collectives cannot read IO tensors -> stage to Internal first)
        nc.sync.dma_start(sstage_d, sblob_d)
        nc.gpsimd.collective_compute(
            "AllGather", mybir.AluOpType.bypass, replica_groups=rg,
            ins=[sstage_d], outs=[sall_d])

        with tc.tile_pool(name="persist", bufs=1) as persist:
            # persistent SBUF
            tab_s = persist.tile([128, ranks, 128], bf)
            loc_a = persist.tile([128, nblk, 128], bf, tag="loca")
            loc_b = persist.tile([128, nblk, 128], bf, tag="locb")
            loc_tabs = [loc_a, loc_b]
            dst_s = persist.tile([128, NT], f32)
            iota_s = persist.tile([128, 128], bf)
            io256_s = persist.tile([128, 256], bf)
            wxi_s = persist.tile([128, N_CONV, 256], bf)
            wxj_s = persist.tile([128, N_CONV, 256], bf)
            wea_s = persist.tile([101, N_CONV, 256], bf)
            g_s = persist.tile([128, N_CONV, 128], f32)
            b_s = persist.tile([128, N_CONV, 128], f32)
            gid_s = persist.tile([128, nblk], f32)
            eps_s = persist.tile([128, 1], f32)
            ones_s = persist.tile([128, 1], f32)

            nc.sync.dma_start(wxi_s[:], wxi_d)
            nc.sync.dma_start(wxj_s[:], wxj_d)
            nc.sync.dma_start(wea_s[:], wea_d)
            nc.sync.dma_start(gid_s[:], gid_d)
            nc.vector.memset(eps_s[:], 1e-5)
            nc.vector.memset(ones_s[:], 1.0)

            # ---------------- init: expansions + tables + edge features --
            with (
                tc.tile_pool(name="initp", bufs=1) as initp,
                tc.tile_pool(name="inits", bufs=3) as inits,
                tc.tile_pool(name="initps", bufs=2, space="PSUM") as initps,
            ):
                noffs_s = initp.tile([101, 1], f32, tag="noffs")
                cfs_s = initp.tile([101, 1], f32, tag="cfs")
                lnr_s = initp.tile([1, N_CONV * 256], f32, tag="lnr")
                ior_s = initp.tile([1, 256], f32, tag="ior")
                one1_s = initp.tile([1, 128], f32, tag="one1")
                on101_s = initp.tile([1, 101], f32, tag="on101")
                dstb_s = initp.tile([128, NT], mybir.dt.int8, tag="dstb")
                nc.sync.dma_start(noffs_s[:], noffs_d)
                nc.sync.dma_start(cfs_s[:], cfs_d)
                nc.sync.dma_start(lnr_s[:], lnr_d)
                nc.sync.dma_start(ior_s[:], ior_d)
                nc.vector.memset(one1_s[:], 1.0)
                nc.vector.memset(on101_s[:], 1.0)
                nc.sync.dma_start(dstb_s[:], dst_d)
                nc.vector.tensor_scalar(
                    out=dst_s[:], in0=dstb_s[:], scalar1=1.0, scalar2=None,
                    op0=mybir.AluOpType.mult)

                # replicate [16, W] index arrays into [128, W] DRAM via SBUF
                # (ixi ships as uint8 and is widened to i16 on the way)
                STW = 2048
                for src_d, dst_x, w, is_u8 in (
                    (ixi_d, ixi_x, S // 16, True),
                    (ixlo_d, ixlo_x, SLO // 16, False),
                    (ixhi_d, ixhi_x, SHI // 16, False),
                    (zall_d, zall_x, slots // 16, False),
                    (zown_d, zown_x, core_slots // 16, False),
                ):
                    for o in range(0, w, STW):
                        ww = min(STW, w - o)
                        if is_u8:
                            st8 = inits.tile([16, STW], mybir.dt.uint8,
                                             tag="ix_st8")
                            nc.sync.dma_start(st8[:, :ww], src_d[:, o:o + ww])
                            st = inits.tile([16, STW], i16, tag="ix_st")
                            nc.vector.tensor_scalar(
                                out=st[:, :ww], in0=st8[:, :ww], scalar1=0,
                                scalar2=None, op0=mybir.AluOpType.add)
                        else:
                            st = inits.tile([16, STW], i16, tag="ix_st")
                            nc.sync.dma_start(st[:, :ww], src_d[:, o:o + ww])
                        for k in range(8):
                            nc.sync.dma_start(
                                dst_x[k * 16:(k + 1) * 16, o:o + ww],
                                st[:, :ww])

                # iota / iota256 built by broadcasting a row over partitions
                pio = initps.tile([128, 512], f32, tag="pio")
                nc.tensor.matmul(pio[:, :256], one1_s[:], ior_s[:],
                                 start=True, stop=True)
                nc.scalar.activation(io256_s[:], pio[:, :256], AF.Copy)
                nc.scalar.activation(iota_s[:], pio[:, :128], AF.Copy)
                # LN gamma/beta broadcast
                for l in range(N_CONV):
                    pln = initps.tile([128, 512], f32, tag="pln")
                    nc.tensor.matmul(pln[:, :256], one1_s[:],
                                     lnr_s[:, l * 256:(l + 1) * 256],
                                     start=True, stop=True)
                    nc.scalar.activation(g_s[:, l, :], pln[:, :128], AF.Copy)
                    nc.scalar.activation(b_s[:, l, :], pln[:, 128:256], AF.Copy)

                # z-indexed gathers build the node tables from the embedding
                zallt = initp.tile([128, slots // 16], i16, tag="zall")
                nc.sync.dma_start(zallt[:], zall_x)
                zownt = initp.tile([128, core_slots // 16], i16, tag="zown")
                nc.sync.dma_start(zownt[:], zown_x)
                PIECE = 4096   # HW limit: big single gathers crash the device
                for o in range(0, slots, PIECE):
                    n = min(PIECE, slots - o)
                    nc.gpsimd.dma_gather(
                        tab_s[:, o // 128:(o + n) // 128, :], ewb_d,
                        zallt[:, o // 16:(o + n) // 16], n, n, 128,
                        transpose=False, single_packet=False)
                x0_t = initp.tile([128, nblk, 128], f32, tag="x0")
                for o in range(0, core_slots, PIECE):
                    n = min(PIECE, core_slots - o)
                    nc.gpsimd.dma_gather(
                        loc_a[:, o // 128:(o + n) // 128, :], ewb_d,
                        zownt[:, o // 16:(o + n) // 16], n, n, 128,
                        transpose=False, single_packet=False)
                    nc.gpsimd.dma_gather(
                        x0_t[:, o // 128:(o + n) // 128, :], ewf_d,
                        zownt[:, o // 16:(o + n) // 16], n, n, 128,
                        transpose=False, single_packet=False)
                nc.sync.dma_start(xinit.rearrange("(r p) c -> p r c", p=128),
                                  x0_t[:])

                # gaussian smearing: ea_k = exp(cfs_k * (d - offs_k)^2);
                # row 100 has cfs=0 -> exp(0)=1 (the conv-bias row).
                # d is broadcast over 101 partitions by a K=1 matmul.
                P = 512
                for off in range(0, S, P):
                    w = min(P, S - off)
                    uti = inits.tile([1, P], i16, tag="uti")
                    nc.sync.dma_start(uti[:, :w], u_d[:, off:off + w])
                    ut = inits.tile([1, P], f32, tag="ut")
                    nc.vector.tensor_scalar(
                        out=ut[:, :w], in0=uti[:, :w],
                        scalar1=float(UQMAX / 32767.0), scalar2=None,
                        op0=mybir.AluOpType.mult)
                    pe_ = initps.tile([101, P], f32, tag="pea")
                    nc.tensor.matmul(pe_[:, :w], on101_s[:], ut[:, :w],
                                     start=True, stop=True)
                    sq = inits.tile([101, P], f32, tag="sq")
                    nc.scalar.activation(sq[:, :w], pe_[:, :w], AF.Square,
                                         bias=noffs_s[:])
                    et = inits.tile([101, P], bf, tag="et")
                    nc.scalar.activation(et[:, :w], sq[:, :w], AF.Exp,
                                         scale=cfs_s[:])
                    nc.sync.dma_start(ea_h[:, off:off + w], et[:, :w])

            if debug_tabs:
                nc.sync.dma_start(dtab_d, tab_s.rearrange("p r c -> p (r c)"))
                nc.sync.dma_start(dloc_d, loc_a.rearrange("p r c -> p (r c)"))

            # ---------------- main conv layers ----------------------------
            with (
                tc.tile_pool(name="gxi", bufs=2) as gxi_p,
                tc.tile_pool(name="glo", bufs=2) as glo_p,
                tc.tile_pool(name="ghi", bufs=2) as ghi_p,
                tc.tile_pool(name="eat", bufs=2) as ea_p,
                tc.tile_pool(name="idx", bufs=2) as idx_p,
                tc.tile_pool(name="small", bufs=3) as small_p,
                tc.tile_pool(name="xio", bufs=2) as xio_p,
                tc.tile_pool(name="stats", bufs=2) as stats_p,
                tc.tile_pool(name="zc", bufs=3, space="PSUM") as zc_p,
                tc.tile_pool(name="agg", bufs=3, space="PSUM") as agg_p,
                tc.tile_pool(name="gsm", bufs=1, space="PSUM") as gsm_p,
            ):
                n_chunks = (nblk + CHUNK - 1) // CHUNK
                tab_flat = tab_s.rearrange("p r c -> p (r c)")
                tab_lo_view = tab_flat[:, : lo_ranks * 128]
                tab_hi_view = tab_flat[:, lo_ranks * 128:]
                gsum0 = gsm_p.tile([128, 128], f32, tag="g0")
                gsum1 = gsm_p.tile([128, 128], f32, tag="g1")

                for layer in range(N_CONV):
                    last = layer == N_CONV - 1
                    xold_src = xinit if layer == 0 else xmast[layer - 1]
                    loc_read = loc_tabs[layer % 2]
                    loc_write = loc_tabs[(layer + 1) % 2]
                    loc_flat = loc_read.rearrange("p r c -> p (r c)")

                    for ch in range(n_chunks):
                        b0 = ch * CHUNK
                        nb = min(CHUNK, nblk - b0)  # blocks in this chunk
                        n_ti = nb * TPB             # xi tiles in chunk
                        n_tl = nb * TL
                        n_th = nb * TH

                        # ---- per-chunk loads -------------------------------
                        ixi_t = idx_p.tile([128, CHUNK * TPB * 8], i16, tag="ixi")
                        ixlo_t = idx_p.tile([128, CHUNK * TL * 8], i16, tag="ixlo")
                        ixhi_t = idx_p.tile([128, CHUNK * TH * 8], i16, tag="ixhi")
                        ea_t = ea_p.tile([101, CHUNK * TPB * 128], bf, tag="ea")
                        c0 = b0 * TPB * 8
                        nc.sync.dma_start(ixi_t[:, :n_ti * 8],
                                          ixi_x[:, c0:c0 + n_ti * 8])
                        nc.sync.dma_start(
                            ixlo_t[:, :n_tl * 8],
                            ixlo_x[:, b0 * TL * 8: b0 * TL * 8 + n_tl * 8])
                        nc.sync.dma_start(
                            ixhi_t[:, :n_th * 8],
                            ixhi_x[:, b0 * TH * 8: b0 * TH * 8 + n_th * 8])
                        nc.sync.dma_start(
                            ea_t[:, :n_ti * 128],
                            ea_h[:, b0 * TPB * 128: (b0 * TPB + n_ti) * 128])

                        # ---- gathers (SBUF-source, transposed, bf16) -------
                        xi_g = gxi_p.tile([128, 1, CHUNK * TPB * 128], bf, tag="xi")
                        lo_g = glo_p.tile([128, 1, CHUNK * TL * 128], bf, tag="lo")
                        hi_g = ghi_p.tile([128, 1, CHUNK * TH * 128], bf, tag="hi")
                        loc_view = loc_flat[:, b0 * 128:(b0 + nb) * 128]
                        nc.gpsimd.dma_gather(
                            xi_g[:, :, :n_ti * 128], loc_view, ixi_t[:, :n_ti * 8],
                            n_ti * 128, n_ti * 128, 128,
                            transpose=True, sbuf_tokens_per_rank=128,
                            sbuf_free_dim_per_rank=256, single_packet=False)
                        nc.gpsimd.dma_gather(
                            lo_g[:, :, :n_tl * 128], tab_lo_view, ixlo_t[:, :n_tl * 8],
                            n_tl * 128, n_tl * 128, 128,
                            transpose=True, sbuf_tokens_per_rank=128,
                            sbuf_free_dim_per_rank=256, single_packet=False)
                        nc.gpsimd.dma_gather(
                            hi_g[:, :, :n_th * 128], tab_hi_view, ixhi_t[:, :n_th * 8],
                            n_th * 128, n_th * 128, 128,
                            transpose=True, sbuf_tokens_per_rank=128,
                            sbuf_free_dim_per_rank=256, single_packet=False)

                        if debug_tabs and layer == 0 and ch == 0:
                            nc.sync.dma_start(dxi_d, xi_g[:, 0, :])
                            nc.sync.dma_start(dlo_d, lo_g[:, 0, :])
                            nc.sync.dma_start(dhi_d, hi_g[:, 0, :])

                        # ---- per-block compute -----------------------------
                        for bi in range(nb):
                            blk = b0 + bi
                            agg = agg_p.tile([128, 128], f32, tag="agg")
                            for t in range(TPB):
                                is_lo = t < TL
                                xi_sl = xi_g[:, 0, (bi * TPB + t) * 128:
                                             (bi * TPB + t + 1) * 128]
                                if is_lo:
                                    xj_sl = lo_g[:, 0, (bi * TL + t) * 128:
                                                 (bi * TL + t + 1) * 128]
                                else:
                                    th = t - TL
                                    xj_sl = hi_g[:, 0, (bi * TH + th) * 128:
                                                 (bi * TH + th + 1) * 128]
                                ea_sl = ea_t[:, (bi * TPB + t) * 128:
                                             (bi * TPB + t + 1) * 128]

                                zc = zc_p.tile([128, 256], f32, tag="zc")
                                nc.tensor.matmul(zc[:], xi_sl, wxi_s[:, layer, :],
                                                 start=True, stop=False)
                                nc.tensor.matmul(zc[:], xj_sl, wxj_s[:, layer, :],
                                                 start=False, stop=False)
                                nc.tensor.matmul(zc[:], ea_sl, wea_s[:, layer, :],
                                                 start=False, stop=True)

                                sel = small_p.tile([128, 128], bf, tag="sel")
                                nc.vector.tensor_scalar(
                                    out=sel[:], in0=iota_s[:],
                                    scalar1=dst_s[:, blk * TPB + t: blk * TPB + t + 1],
                                    scalar2=None, op0=mybir.AluOpType.is_equal)

                                # zc holds [-z1 | z2] (z1-half weights
                                # sign-flipped on host).
                                # msg = softplus(z2) * sigmoid(z1)
                                #     = ln(1+e^{z2}) / (1 + e^{-z1})
                                ez = small_p.tile([128, 256], f32, tag="ez")
                                nc.scalar.activation(ez[:], zc[:], AF.Exp)
                                sp = small_p.tile([128, 128], bf, tag="sp")
                                nc.scalar.activation(sp[:], ez[:, 128:256],
                                                     AF.Ln, bias=ones_s[:])
                                u1 = small_p.tile([128, 128], f32, tag="u1")
                                nc.vector.tensor_scalar(
                                    out=u1[:], in0=ez[:, 0:128], scalar1=1.0,
                                    scalar2=None, op0=mybir.AluOpType.add)
                                rcp = small_p.tile([128, 128], f32, tag="rcp")
                                nc.vector.reciprocal(rcp[:], u1[:])
                                msg = small_p.tile([128, 128], bf, tag="msg")
                                nc.vector.tensor_mul(msg[:], sp[:], rcp[:])

                                nc.tensor.matmul(agg[:], sel[:], msg[:],
                                                 start=(t == 0), stop=(t == TPB - 1))
                                if debug_tabs and layer == 0 and blk == 0 and t == 0:
                                    nc.sync.dma_start(dmsg_d, msg[:])

                            # ---- block epilogue: LN + residual + softplus --
                            if debug_tabs and layer == 0 and blk == 0:
                                dag = xio_p.tile([128, 128], f32, tag="dag")
                                nc.scalar.activation(dag[:], agg[:], AF.Copy)
                                nc.sync.dma_start(dagg_d, dag[:])
                            xold = xio_p.tile([128, 128], f32, tag="xold")
                            nc.sync.dma_start(
                                xold[:], xold_src[blk * 128:(blk + 1) * 128, :])

                            st = stats_p.tile([128, 6], f32, tag="bn")
                            nc.vector.bn_stats(out=st[:], in_=agg[:])
                            mv = stats_p.tile([128, 2], f32, tag="mv")
                            nc.vector.bn_aggr(out=mv[:], in_=st[:])
                            # rstd = exp(-0.5 * ln(var + eps))
                            lnv = stats_p.tile([128, 1], f32, tag="lnv")
                            nc.scalar.activation(lnv[:], mv[:, 1:2], AF.Ln,
                                                 bias=eps_s[:])
                            rstd = stats_p.tile([128, 1], f32, tag="rstd")
                            nc.scalar.activation(rstd[:], lnv[:], AF.Exp,
                                                 scale=-0.5)

                            xn = xio_p.tile([128, 128], f32, tag="xn")
                            nc.vector.tensor_scalar(
                                out=xn[:], in0=agg[:], scalar1=mv[:, 0:1],
                                scalar2=rstd[:], op0=mybir.AluOpType.subtract,
                                op1=mybir.AluOpType.mult)
                            nc.vector.tensor_mul(xn[:], xn[:], g_s[:, layer, :])
                            nc.vector.tensor_add(xn[:], xn[:], b_s[:, layer, :])
                            nc.vector.tensor_add(xn[:], xn[:], xold[:])

                            # softplus(xn) = ln(1 + e^{xn})
                            exn = xio_p.tile([128, 128], f32, tag="exn")
                            nc.scalar.activation(exn[:], xn[:], AF.Exp)
                            xnew = xio_p.tile([128, 128], f32, tag="xnew")
                            nc.scalar.activation(xnew[:], exn[:], AF.Ln,
                                                 bias=ones_s[:])
                            if not last:
                                # bf16 copy into next layer's local gather table
                                nc.scalar.activation(loc_write[:, blk, :],
                                                     xnew[:], AF.Copy)
                                nc.sync.dma_start(
                                    xmast[layer][blk * 128:(blk + 1) * 128, :],
                                    xnew[:])
                            else:
                                # pooled per-graph sums: gsum[g,:] += x[slot,:]
                                gsel = xio_p.tile([128, 256], f32, tag="gsel")
                                nc.vector.tensor_scalar(
                                    out=gsel[:], in0=io256_s[:],
                                    scalar1=gid_s[:, blk:blk + 1],
                                    scalar2=None, op0=mybir.AluOpType.is_equal)
                                nc.tensor.matmul(gsum0[:], gsel[:, 0:128],
                                                 xnew[:], start=(blk == 0),
                                                 stop=(blk == nblk - 1))
                                nc.tensor.matmul(gsum1[:], gsel[:, 128:256],
                                                 xnew[:], start=(blk == 0),
                                                 stop=(blk == nblk - 1))

                    # ---- exchange (layers 0,1): slice -> AllGather -> table
                    if not last:
                        nc.sync.dma_start(
                            xout[layer].rearrange("(r p) c -> p r c", p=128),
                            loc_write[:])
                        nc.gpsimd.collective_compute(
                            "AllGather", mybir.AluOpType.bypass,
                            replica_groups=rg,
                            ins=[xout[layer][:]], outs=[xall[layer][:]])
                        nc.sync.dma_start(
                            tab_s[:],
                            xall[layer].rearrange("(r p) c -> p r c", p=128))

                # ---- write pooled output ---------------------------------
                gsb = xio_p.tile([128, 256], f32, tag="gsb")
                nc.scalar.activation(gsb[:, 0:128], gsum0[:], AF.Copy)
                nc.scalar.activation(gsb[:, 128:256], gsum1[:], AF.Copy)
                nc.sync.dma_start(gfull_d[0:128, :], gsb[:, 0:128])
                nc.sync.dma_start(gfull_d[128:256, :], gsb[:, 128:256])
                nc.gpsimd.collective_compute(
                    "ReduceScatter", mybir.AluOpType.add, replica_groups=rg,
                    ins=[gfull_d], outs=[gpart_d])
                nc.sync.dma_start(gsum_d, gpart_d)

    nc.compile()
    return nc


# --------------------------------------------------------------------------
# Host preprocessing
# --------------------------------------------------------------------------

def _softplus(x):
    return np.log1p(np.exp(-np.abs(x))) + np.maximum(x, 0.0)


def _snake_slots(n, n_bins):
    """Slot offsets (bin*128 + round) for n items dealt snake-wise, in the
    order of the sorted item list."""
    idx = np.arange(n)
    r = idx // n_bins
    k = idx % n_bins
    bins = np.where(r % 2 == 0, k, n_bins - 1 - k)
    return bins * 128 + r


def _wrap16(arr):
    # [n] int16 -> [16, n/16], idx i at (i%16, i//16)
    return np.ascontiguousarray(arr.reshape(-1, 16).T)


def preprocess(z, R, edge_index, batch, embedding, emb_w, emb_b, conv_w, conv_b,
               ln_g, ln_b, n_nodes=N_NODES, n_cores=N_CORES, nblk=NBLK,
               lo_slots=LO_SLOTS, edge_d=EDGE_D, cutoff=CUTOFF):
    blocks = n_cores * nblk
    slots = blocks * 128
    core_slots = nblk * 128
    lo_blocks = lo_slots // 128
    n_edges = edge_index.shape[1]
    src = np.asarray(edge_index[0], np.int64)
    dst = np.asarray(edge_index[1], np.int64)

    # edge distances on host (smearing runs on device)
    Rf = np.asarray(R, np.float32)
    d = np.linalg.norm(Rf[src] - Rf[dst], axis=-1)  # [E] f32

    # node permutation: balance per-block in-degrees; L = orig nodes < lo_slots
    islo_e = src < lo_slots
    a = np.bincount(dst[islo_e], minlength=n_nodes)
    b = np.bincount(dst[~islo_e], minlength=n_nodes)
    w = a + b
    # L-nodes -> slots [0, lo_slots); rest -> [lo_slots, slots)
    ordL = np.argsort(-w[:lo_slots], kind="stable")
    ordH = np.argsort(-w[lo_slots:], kind="stable") + lo_slots
    perm = np.full(n_nodes, -1, np.int64)
    perm[ordL] = _snake_slots(ordL.size, lo_blocks)
    perm[ordH] = _snake_slots(ordH.size, blocks - lo_blocks) + lo_slots
    assert perm.min() >= 0

    es, ed = perm[src], perm[dst]
    blk = ed // 128

    lo_cnt = np.bincount(blk[islo_e], minlength=blocks)
    hi_cnt = np.bincount(blk[~islo_e], minlength=blocks)
    TL = int(-(-lo_cnt.max() // 128))
    TH = int(-(-hi_cnt.max() // 128))
    TPB = TL + TH
    S = nblk * TPB * 128

    # edge slot assignment: within block, lows first then highs
    key = blk * 2 + (~islo_e).astype(np.int64)
    eorder = np.argsort(key, kind="stable")
    ks = key[eorder]
    # position within each (block, half) run
    runstart = np.r_[0, np.flatnonzero(np.diff(ks)) + 1]
    runid = np.zeros(n_edges, np.int64)
    runid[runstart[1:]] = 1
    runid = np.cumsum(runid)
    pos = np.arange(n_edges) - runstart[runid]
    eb = ks // 2
    ehalf = ks % 2
    base = eb * TPB * 128 + ehalf * (TL * 128)
    eslot_g = base + pos                       # global edge slot (per full graph)
    # per-core arrays
    core_of = eb // nblk
    eslot = eslot_g - core_of * (nblk * TPB * 128)

    ixi = np.zeros((n_cores, S), np.int16)
    ixlo = np.zeros((n_cores, nblk * TL * 128), np.int16)
    ixhi = np.zeros((n_cores, nblk * TH * 128), np.int16)
    dstv = np.full((n_cores, nblk * TPB, 128), -1.0, np.float32)
    u = np.zeros((n_cores, 1, S), np.float32)

    e_src = es[eorder]
    e_dst = ed[eorder]
    e_lo = ehalf == 0
    d_o = d[eorder]

    for c in range(n_cores):
        m = core_of == c
        sl = eslot[m]
        # xi: dst local to the chunk's 2-block view (fits uint8)
        dloc = (e_dst[m] - c * core_slots) % (CHUNK * 128)
        ixi[c][sl] = dloc.astype(np.int16)
        assert dloc.max(initial=0) < 256
        # xj
        mlo = m & e_lo
        mhi = m & ~e_lo
        slo_ = eslot[mlo]
        bb = slo_ // (TPB * 128)
        off = slo_ - bb * (TPB * 128)
        ixlo[c][bb * TL * 128 + off] = e_src[mlo].astype(np.int16)
        shi_ = eslot[mhi]
        bb = shi_ // (TPB * 128)
        off = shi_ - bb * (TPB * 128) - TL * 128
        ixhi[c][bb * TH * 128 + off] = (e_src[mhi] - lo_slots).astype(np.int16)
        # dst one-hot value, edge distances
        dstv[c].reshape(-1)[sl] = (e_dst[m] % 128).astype(np.float32)
        u[c, 0, sl] = d_o[m].astype(np.float32)

    # z tables (slot -> atom type; empty slots -> 100 which maps to a 0 row)
    zslot = np.full(slots, 100, np.int16)
    zslot[perm] = np.asarray(z, np.int16)
    # graph-id per slot (empty -> -1, excluded from pooling)
    gslot = np.full(slots, -1.0, np.float32)
    gslot[perm] = np.asarray(batch, np.float32)

    # embedding rows
    EWf = np.zeros((128, 128), np.float32)
    EWf[:100] = (np.asarray(embedding, np.float32)
                 @ np.asarray(emb_w, np.float32)
                 + np.asarray(emb_b, np.float32))
    EWb = EWf.astype(BF16)

    # conv weights; z1-half output columns sign-flipped so the device computes
    # [-z1 | z2] and can use exp/ln-only activations (one act table)
    cw = np.asarray(conv_w, np.float32).copy()
    cb = np.asarray(conv_b, np.float32).copy()
    cw[:, :, :128] *= -1.0
    cb[:, :128] *= -1.0
    wxi = np.ascontiguousarray(cw[:, :128, :].transpose(1, 0, 2)).astype(BF16)
    wxj = np.ascontiguousarray(cw[:, 128:256, :].transpose(1, 0, 2)).astype(BF16)
    wea = np.concatenate([cw[:, 256:, :], cb[:, None, :]], axis=1)
    wea = np.ascontiguousarray(wea.transpose(1, 0, 2)).astype(BF16)

    # LN gamma/beta rows
    lnr = np.concatenate(
        [np.concatenate([np.asarray(ln_g, np.float32)[l],
                         np.asarray(ln_b, np.float32)[l]])
         for l in range(cw.shape[0])])[None, :]

    # smearing: ea_k = exp(cfs_k * (d - offs_k)^2); cfs[100]=0 -> bias row 1
    offs = np.linspace(0.0, cutoff, edge_d, dtype=np.float32)
    coeff = np.float32(-0.5 / (offs[1] - offs[0]) ** 2)
    noffs = np.zeros((101, 1), np.float32)
    noffs[:edge_d, 0] = -offs
    cfs = np.zeros((101, 1), np.float32)
    cfs[:edge_d, 0] = coeff

    ior = np.arange(256, dtype=np.float32)[None, :]

    # ---- pack blobs ------------------------------------------------------
    uents, sents, BU, BS, BS8 = _layout(TL, TH, nblk, ranks=blocks,
                                        n_cores=n_cores)

    def pack(ents, arrays, nbytes):
        blob = np.zeros(nbytes // 2, np.int16)
        bv = blob.view(np.uint8)
        for name, (off, shape, dt_) in ents.items():
            a = np.ascontiguousarray(arrays[name])
            assert a.shape == tuple(shape) and a.dtype == np.dtype(dt_), \
                (name, a.shape, shape, a.dtype, dt_)
            bv[off:off + a.nbytes] = a.view(np.uint8).ravel()
        return blob

    sblob = pack(sents, {
        "zall": _wrap16(zslot), "ewb": EWb, "ewf": EWf,
        "wxi": wxi, "wxj": wxj, "wea": wea,
        "lnr": lnr, "ior": ior, "noffs": noffs, "cfs": cfs,
    }, BS)

    in_maps = []
    for c in range(n_cores):
        sl0 = c * core_slots
        uq = np.round(np.minimum(u[c], UQMAX) * (32767.0 / UQMAX)
                      ).astype(np.int16)
        ublob = pack(uents, {
            "u": uq,
            "ixi": _wrap16(ixi[c]).astype(np.uint8),
            "ixlo": _wrap16(ixlo[c]),
            "ixhi": _wrap16(ixhi[c]),
            "zown": _wrap16(zslot[sl0:sl0 + core_slots]),
            "dstv": np.ascontiguousarray(
                dstv[c].transpose(1, 0)).astype(np.int8),
            "gid": np.ascontiguousarray(
                gslot[sl0:sl0 + core_slots].reshape(nblk, 128).T),
        }, BU)
        in_maps.append({
            "ublob": ublob,
            "sblob": sblob[c * BS8 // 2:(c + 1) * BS8 // 2],
        })
    return in_maps, TL, TH


# --------------------------------------------------------------------------
# execution: cached jitted SPMD runner (PJRT via bass2jax custom call)
# --------------------------------------------------------------------------

class _Results:
    """Minimal stand-in for BassKernelResults (test.py reads exec_time_ns)."""

    def __init__(self, results):
        self.results = results
        self.exec_time_ns = None


class _Runner:
    """Compile once, then run full numpy in_maps -> numpy outputs.

    Reimplements bass_utils.run_bass_kernel_spmd's axon path with a CACHED
    jitted callable: a fresh jax.jit per call costs ~2.5 s of re-trace/
    re-lowering for an identical program. Each run() still performs the full
    host->device transfer of every input, the NEFF execution on all 8 cores,
    and the device->host readback of the outputs.
    """

    def __init__(self, nc, n_cores):
        import jax
        from jax.sharding import Mesh, PartitionSpec
        from jax.experimental.shard_map import shard_map
        from concourse import bass2jax

        bass2jax.install_neuronx_cc_hook()
        self.nc = nc
        self.n_cores = n_cores
        partition_name = (nc.partition_id_tensor.name
                          if nc.partition_id_tensor else None)
        in_names, out_names, out_avals, zero_outs = [], [], [], []
        for alloc in nc.m.functions[0].allocations:
            if not isinstance(alloc, mybir.MemoryLocationSet):
                continue
            name = alloc.memorylocations[0].name
            if alloc.kind == "ExternalInput":
                if name != partition_name:
                    in_names.append(name)
            elif alloc.kind == "ExternalOutput":
                shape = tuple(alloc.tensor_shape)
                dtype = mybir.dt.np(alloc.dtype)
                out_names.append(name)
                out_avals.append(jax.core.ShapedArray(shape, dtype))
                zero_outs.append(np.zeros((n_cores * shape[0], *shape[1:]),
                                          dtype))
        self.in_names = in_names
        self.out_names = out_names
        self.out_shapes = [tuple(a.shape) for a in out_avals]
        self.zero_outs = zero_outs
        n_params = len(in_names)
        all_in = in_names + out_names + (
            [partition_name] if partition_name else [])

        def _body(*args):
            operands = list(args)
            if partition_name is not None:
                operands.append(bass2jax.partition_id_tensor())
            outs = bass2jax._bass_exec_p.bind(
                *operands, out_avals=tuple(out_avals),
                in_names=tuple(all_in), out_names=tuple(out_names),
                lowering_input_output_aliases=(),
                sim_require_finite=True, sim_require_nnan=True, nc=nc)
            return tuple(outs)

        devs = jax.devices()[:n_cores]
        assert len(devs) == n_cores
        mesh = Mesh(np.asarray(devs), ("core",))
        n_outs = len(out_avals)
        self._fn = jax.jit(
            shard_map(_body, mesh=mesh,
                      in_specs=(PartitionSpec("core"),) * (n_params + n_outs),
                      out_specs=(PartitionSpec("core"),) * n_outs,
                      check_rep=False),
            donate_argnums=tuple(range(n_params, n_params + n_outs)),
            keep_unused=True)
    def run(self, in_maps):
        concat_in = [
            np.concatenate([np.asarray(m[n]) for m in in_maps], axis=0)
            for n in self.in_names]
        outs = self._fn(*concat_in, *self.zero_outs)
        n = self.n_cores
        return _Results([
            {name: np.asarray(outs[i]).reshape(n, *self.out_shapes[i])[c]
             for i, name in enumerate(self.out_names)}
            for c in range(n)])


def kernel(z, R, edge_index, batch, embedding, emb_w, emb_b, conv_w, conv_b,
           ln_g, ln_b, cfc_w, cfc_b, fc_w, fc_b, out_w, out_b):
    in_maps, TL, TH = preprocess(
        z, R, edge_index, batch, embedding, emb_w, emb_b, conv_w, conv_b,
        ln_g, ln_b)

    key = (TL, TH)
    if key not in _NC_CACHE:
        nc = build_nc(TL, TH)
        _NC_CACHE[key] = _Runner(nc, N_CORES)
    runner = _NC_CACHE[key]

    res = runner.run(in_maps)
    global LAST_RESULTS, LAST_RERUN_S, LAST_RUN
    LAST_RUN = (runner, in_maps)
    LAST_RESULTS = res
    if _os.environ.get("KERNEL_RERUN", "1") != "0":
        import time as _time
        t0 = _time.time()
        runner.run(in_maps)
        LAST_RERUN_S = _time.time() - t0

    gs = np.concatenate([res.results[c]["gsum"] for c in range(N_CORES)],
                        axis=0)  # [256, 128] fully-summed (reduce-scattered)

    batch = np.asarray(batch, np.int64)
    cnts = np.bincount(batch, minlength=N_GRAPHS).astype(np.float32)
    mol = gs / np.maximum(cnts, 1.0)[:, None]

    h = _softplus(mol @ np.asarray(cfc_w, np.float32) + np.asarray(cfc_b, np.float32))
    for l in range(np.asarray(fc_w).shape[0]):
        h = _softplus(h @ np.asarray(fc_w[l], np.float32)
                      + np.asarray(fc_b[l], np.float32))
    out = h @ np.asarray(out_w, np.float32) + np.asarray(out_b, np.float32)
    return out.astype(np.float32)

